# revision 1
# baseline (speedup 1.0000x reference)
"""Trainium2 Bass kernel for BrainFunctionalConnectivityFeatureExtractionModule.

Math (per batch b, all f32):
    w    = relu(adj + adj_bias)                       (16,16)
    d    = 1/sqrt(sum(w, axis=1) + 1e-5)              (16,)
    lap  = I - d[:,None] * w * d[None,:]              (16,16)
    t1   = lap @ x[b]                                 (16,256)
    cp   = interleave(ones, t1)                       (16,512)
    h    = relu(brelu_bias + cp @ cheb_w)             (16,64)
    out  = h @ fc_w.T + fc_b                          (16,387)

Since the even interleaved lanes of cp are all-ones,
    cp @ cheb_w = t1 @ cheb_w[1::2] + sum(cheb_w[0::2], axis=0)
so the whole module collapses to three chained matmuls + relu:
    h   = relu(t1 @ W1 + bias_h),   W1 = cheb_w[1::2]  (256,64)
    out = h @ fc_w.T + fc_b

Device mapping: pure data parallel over 8 cores, B=8192 -> 1024 batches/core,
ROWS = 1024*16 = 16384 (b,e)-rows per core, processed in 512-row macro tiles
of 4 x 128-row sub-tiles (sub-tile = 8 full 16-node graphs).

The awkward step is t1 = lap @ x[b]: the E-contraction runs along the SBUF
partition axis of x.  We fold the 16x16 lap mix into a transposing matmul:
      t1T[c, n] = x_sub[:, c_chunk].T @ (I_8 (x) lap^T)      [n = row in sub]
which lands t1 TRANSPOSED ([C on partitions, rows free]) -- exactly the
layout the W1 matmul wants.  Stage 2: hT[64, 512] = sum_k W1_k^T @ t1T_k.
Stage 3: out[128, 388pad] = hT_slice.T @ fc_wT; +fc_b is fused into the
PSUM->SBUF copy as a tensor_add against a partition-replicated fc_b tile.
All matmul inputs are bf16 (PSUM accumulation is f32): fp32/fp32r matmuls
hit a 2-4x slower datapath on trn2 and do not engage the PE clock-gate
release (measured: whole kernel stuck at K=4/8, 1.2 GHz).  x is loaded as
f32 (plain HWDGE DMA; the SWDGE cast-DMA path hung full-scale 8-core runs)
and cast to bf16 on-chip, split between the vector and scalar engines.
"""

import numpy as np
from contextlib import ExitStack

B, E, C, H, OUT = 8192, 16, 256, 64, 387
NCORES = 8
ROWS = (B // NCORES) * E        # 16384 rows per core
NS = 4                          # sub-tiles per macro tile
TR = 128 * NS                   # 512 macro-tile rows
NT = ROWS // TR                 # 32 macro tiles per core
KC = C // 128                   # 2 contraction chunks of 128
OUTP = OUT + 1                  # fc matmul N padded even

_cache = {}


def _build_module():
    import concourse.tile as tile
    from concourse import bacc, mybir

    f32 = mybir.dt.float32
    bf16 = mybir.dt.bfloat16
    Relu = mybir.ActivationFunctionType.Relu

    nc = bacc.Bacc("TRN2", target_bir_lowering=False, debug=False,
                   num_devices=NCORES)

    x_d = nc.dram_tensor("x", (ROWS, C), f32, kind="ExternalInput").ap()
    r_d = nc.dram_tensor("r", (128, 128), bf16, kind="ExternalInput").ap()
    w1_d = nc.dram_tensor("w1", (KC, 128, H), bf16, kind="ExternalInput").ap()
    bh_d = nc.dram_tensor("bh", (H, 1), f32, kind="ExternalInput").ap()
    fcw_d = nc.dram_tensor("fcw", (H + 1, OUTP), bf16, kind="ExternalInput").ap()
    o_d = nc.dram_tensor("o", (ROWS, OUT), f32, kind="ExternalOutput").ap()

    with tile.TileContext(nc) as tc:
        with ExitStack() as ctx:
            consts = ctx.enter_context(tc.tile_pool(name="consts", bufs=1))
            xp = ctx.enter_context(tc.tile_pool(name="xp", bufs=3))
            xbp = ctx.enter_context(tc.tile_pool(name="xbp", bufs=3))
            t1sp = ctx.enter_context(tc.tile_pool(name="t1sp", bufs=3))
            hp = ctx.enter_context(tc.tile_pool(name="hp", bufs=3))
            op = ctx.enter_context(tc.tile_pool(name="op", bufs=3))
            t1pp = ctx.enter_context(tc.tile_pool(name="t1pp", bufs=2, space="PSUM"))
            hpp = ctx.enter_context(tc.tile_pool(name="hpp", bufs=2, space="PSUM"))
            opp = ctx.enter_context(tc.tile_pool(name="opp", bufs=2, space="PSUM"))

            r_sb = consts.tile([128, 128], bf16)
            nc.sync.dma_start(r_sb, r_d)
            w1_sb = consts.tile([128, KC, H], bf16)
            nc.sync.dma_start(w1_sb, w1_d.rearrange("k p h -> p k h"))
            bh_sb = consts.tile([H, 1], f32)
            nc.sync.dma_start(bh_sb, bh_d)
            fcw_sb = consts.tile([H + 1, OUTP], bf16)
            nc.sync.dma_start(fcw_sb, fcw_d)

            # x: row l of macro t lives at sub-tile l//128, partition l%128
            xv = x_d.rearrange("(t s p) c -> t p s c", p=128, s=NS)
            # out: row l at partition l//4, slot l%4 -> 6KB contiguous runs
            ov = o_d.rearrange("(t p s) o -> t p s o", p=128, s=NS)

            for t in range(NT):
                x32_sb = xp.tile([128, NS, C], f32)
                nc.sync.dma_start(x32_sb, xv[t])
                x_sb = xbp.tile([128, NS, C], bf16)
                nc.vector.tensor_copy(x_sb, x32_sb)

                # stage 1: t1T[c, s*128+n] = x[:, s, c_chunk].T @ (I8 (x) lapT)
                t1_ps = t1pp.tile([128, KC, TR], f32)
                for k in range(KC):
                    for s in range(NS):
                        nc.tensor.matmul(
                            t1_ps[:, k, s * 128:(s + 1) * 128],
                            lhsT=x_sb[:, s, k * 128:(k + 1) * 128],
                            rhs=r_sb,
                        )
                t1_sb = t1sp.tile([128, KC, TR], bf16)
                nc.vector.tensor_copy(t1_sb[:, 0, :], t1_ps[:, 0, :])
                nc.scalar.copy(t1_sb[:, 1, :], t1_ps[:, 1, :])

                # stage 2: hT[h, n] = sum_k W1_k.T @ t1T_k
                h_ps = hpp.tile([H, TR], f32)
                for k in range(KC):
                    nc.tensor.matmul(
                        h_ps,
                        lhsT=w1_sb[:, k, :],
                        rhs=t1_sb[:, k, :],
                        start=(k == 0),
                        stop=(k == KC - 1),
                    )
                hT_sb = hp.tile([H + 1, TR], bf16)
                nc.gpsimd.memset(hT_sb[H:H + 1, :], 1.0)
                nc.scalar.activation(hT_sb[0:H, :], h_ps, Relu, bias=bh_sb)

                # stage 3: slot s covers rows l = 4p + s (hT cols s::4)
                o_sb = op.tile([128, NS, OUT], f32)
                hT_v = hT_sb.rearrange("h (n s) -> h s n", s=NS)
                for s in range(NS):
                    o_ps = opp.tile([128, OUTP], f32)
                    nc.tensor.matmul(
                        o_ps,
                        lhsT=hT_v[:, s, :],
                        rhs=fcw_sb,
                    )
                    if s % 2 == 0:
                        nc.vector.tensor_copy(o_sb[:, s, :], o_ps[:, 0:OUT])
                    else:
                        nc.scalar.copy(o_sb[:, s, :], o_ps[:, 0:OUT])
                nc.sync.dma_start(ov[t], o_sb)

    nc.finalize()
    return nc


def _host_prep(adj, adj_bias, cheb_w, brelu_bias, fc_w, fc_b):
    import ml_dtypes

    bf = ml_dtypes.bfloat16
    adj = np.asarray(adj, np.float32)
    w = np.maximum(adj + np.float32(adj_bias.reshape(())), 0.0)
    d = 1.0 / np.sqrt(w.sum(axis=1) + np.float32(1e-5))
    lap = np.eye(E, dtype=np.float32) - d[:, None] * w * d[None, :]

    # r = I_8 (x) lap^T : [p = b*16+j, n = b*16+i] -> lap[i, j]
    r = np.kron(np.eye(128 // E, dtype=np.float32), lap.T)

    cheb_w = np.asarray(cheb_w, np.float32)
    w1 = np.ascontiguousarray(cheb_w[1::2, :]).reshape(KC, 128, H)
    bias_h = (cheb_w[0::2, :].sum(axis=0)
              + np.asarray(brelu_bias, np.float32).reshape(H))
    fcw = np.zeros((H + 1, OUTP), np.float32)
    fcw[:H, :OUT] = np.asarray(fc_w, np.float32).T
    fcw[H, :OUT] = np.asarray(fc_b, np.float32)
    return {
        "r": r.astype(bf),
        "w1": np.ascontiguousarray(w1).astype(bf),
        "bh": bias_h.reshape(H, 1).astype(np.float32),
        "fcw": fcw.astype(bf),
    }


def _run(inputs, trace=False, **kw):
    from concourse import bass_utils

    if "nc" not in _cache:
        _cache["nc"] = _build_module()
    nc = _cache["nc"]

    x = np.asarray(inputs["x"], np.float32)
    weights = _host_prep(inputs["adj"], inputs["adj_bias"], inputs["cheb_w"],
                         inputs["brelu_bias"], inputs["fc_w"], inputs["fc_b"])

    shards = x.reshape(NCORES, ROWS, C)
    in_maps = [dict(weights, x=np.ascontiguousarray(shards[c]))
               for c in range(NCORES)]

    res = bass_utils.run_bass_kernel_spmd(
        nc, in_maps, core_ids=list(range(NCORES)), trace=trace, **kw)

    out = np.concatenate(
        [res.results[c]["o"].reshape(B // NCORES, E, OUT)
         for c in range(NCORES)], axis=0)
    return out, res


def kernel(**inputs) -> np.ndarray:
    out, _ = _run(inputs, trace=False)
    return out

